# revision 14
# baseline (speedup 1.0000x reference)
"""Trainium2 Bass kernel for the EquivariantMLPBlock problem.

Math (per row n of x [N, 1920]):
  s = x[:, :512]; v = x[:, 512:1280] as [256, 3]; t = x[:, 1280:] as [128, 5]
  s_out = s @ W0 / sqrt(512)                     -> [896]
  v_out[o, m] = sum_i v[i, m] W1[i, o] / sqrt(256)
  t_out[o, m] = sum_i t[i, m] W2[i, o] / sqrt(128)
  out = [leaky_relu(s_out[:512]),
         (v_out * sigmoid(s_out[512:768])[:, None]).flat,
         (t_out * sigmoid(s_out[768:])[:, None]).flat]

Strategy: data-parallel over rows (8 cores). On the host, the feature
axis is permuted to a "grouped" layout (each m-component of v/t made
contiguous) and x is transposed so features sit on SBUF partitions,
making every matmul a plain weight-stationary PE matmul with rows
streaming on the free axis. The DRAM image is packed per SBUF partition
([p, tile, chunk, col]) so each DMA moves one long contiguous run per
partition (13KB packets instead of 0.9KB rows). Device I/O and matmul
operands are fp16 (halves the DMA bytes; PE runs fp16 at full rate) with
fp32 PSUM accumulation; gating/sigmoid/leaky-relu all run in fp32 on
ACT/DVE. Measured end-to-end error vs the fp32 reference is ~5e-4 of
the output scale (rms ~3.7e-4). Set _IO16=False for fp32r matmuls
(~1.5e-4, ~294us) or _IO16=False,_F32R=False for full fp32 (~1e-6,
~499us). Gate blocks are computed first (their sigmoid feeds every
gating mul), leaky-relu blocks last; outputs drain via the idle GpSimd
DMA queue so stores never block input prefetch on the Sync ring.
Output comes back transposed+grouped and is un-permuted on the host.
"""
import sys
sys.path.insert(0, '/opt/trn_rl_repo')

import numpy as np
from contextlib import ExitStack

D = 1920                 # feature dim
NCHUNK = D // 128        # 15 partition chunks
N_FULL = 50000
N_CORES = 8
NC_PAD = 6272            # rows per core after padding: 8*6272 = 50176
B = 448                  # rows per column-tile (PE moving free dim)
NT = NC_PAD // B         # 14 column tiles

_TRACE = False           # set by test harness to capture an NTFF profile
_LAST_RESULTS = None     # stashed BassKernelResults for the harness
_F32R = True             # fp32r matmuls (TF32-like, ~1.5e-4 rel err, 4x PE speed)
_IO16 = True             # fp16 device I/O + fp16 matmul operands (halves DMA bytes)


def _perm():
    # grouped feature order: [s(512) | v m=0 (256) | v m=1 | v m=2 | t m=0 (128) ... t m=4]
    p = list(range(512))
    for m in range(3):
        p += [512 + i * 3 + m for i in range(256)]
    for m in range(5):
        p += [1280 + i * 5 + m for i in range(128)]
    return np.asarray(p, dtype=np.int64)


_compiled_nc = None


def _build():
    global _compiled_nc
    if _compiled_nc is not None:
        return _compiled_nc
    import concourse.tile as tile
    from concourse import bacc, mybir
    from concourse.alu_op_type import AluOpType

    f32 = mybir.dt.float32
    f16 = mybir.dt.float16
    fio = f16 if _IO16 else f32
    fmm = f16 if _IO16 else (mybir.dt.float32r if _F32R else f32)
    AFT = mybir.ActivationFunctionType

    nc = bacc.Bacc("TRN2", target_bir_lowering=False, debug=False)
    # packed layouts: element (p, it, c, j) = feature (c*128+p), row (it*B+j)
    xt = nc.dram_tensor("xt", [128, NT, NCHUNK, B], fio, kind="ExternalInput").ap()
    w0 = nc.dram_tensor("w0", [512, 896], fio, kind="ExternalInput").ap()
    w1 = nc.dram_tensor("w1", [256, 256], fio, kind="ExternalInput").ap()
    w2 = nc.dram_tensor("w2", [128, 128], fio, kind="ExternalInput").ap()
    out = nc.dram_tensor("out", [128, NT, NCHUNK, B], fio, kind="ExternalOutput").ap()

    with tile.TileContext(nc) as tc:
        with ExitStack() as ctx:
            wpool = ctx.enter_context(tc.tile_pool(name="w", bufs=1))
            xpool = ctx.enter_context(tc.tile_pool(name="x", bufs=6))
            gpool = ctx.enter_context(tc.tile_pool(name="g", bufs=3))
            opool = ctx.enter_context(tc.tile_pool(name="o", bufs=6))
            pspool = ctx.enter_context(tc.tile_pool(name="ps", bufs=8, space="PSUM"))

            w0t = wpool.tile([128, 4, 896], fmm)
            for k in range(4):
                nc.sync.dma_start(w0t[:, k, :], w0[k * 128:(k + 1) * 128, :].bitcast(fmm))
            w1t = wpool.tile([128, 2, 256], fmm)
            for k in range(2):
                nc.sync.dma_start(w1t[:, k, :], w1[k * 128:(k + 1) * 128, :].bitcast(fmm))
            w2t = wpool.tile([128, 128], fmm)
            nc.sync.dma_start(w2t[:], w2[:, :].bitcast(fmm))

            for it in range(NT):
                xtile = xpool.tile([128, NCHUNK, B], fmm)
                nc.sync.dma_start(xtile[:, :, :], xt[:, it, :, :].bitcast(fmm))
                otile = opool.tile([128, NCHUNK, B], fio)
                gtile = gpool.tile([128, 3, B], f32)

                # gate blocks first: their sigmoid output feeds every v/t
                # gating mul, so they head the per-tile critical path
                for ob in range(4, 7):
                    ps = pspool.tile([128, B], f32)
                    for k in range(4):
                        nc.tensor.matmul(
                            ps[:],
                            w0t[:, k, ob * 128:(ob + 1) * 128],
                            xtile[:, k, :],
                            start=(k == 0),
                            stop=(k == 3),
                        )
                    nc.scalar.activation(gtile[:, ob - 4, :], ps[:], AFT.Sigmoid)

                # 1o block: 3 m-components, each [256 -> 256]
                for m in range(3):
                    for ob in range(2):
                        ps = pspool.tile([128, B], f32)
                        for k in range(2):
                            nc.tensor.matmul(
                                ps[:],
                                w1t[:, k, ob * 128:(ob + 1) * 128],
                                xtile[:, 4 + 2 * m + k, :],
                                start=(k == 0),
                                stop=(k == 1),
                            )
                        nc.vector.tensor_mul(otile[:, 4 + 2 * m + ob, :], ps[:], gtile[:, ob, :])

                # 2e block: 5 m-components, each [128 -> 128]
                for m in range(5):
                    ps = pspool.tile([128, B], f32)
                    nc.tensor.matmul(ps[:], w2t[:], xtile[:, 10 + m, :], start=True, stop=True)
                    nc.vector.tensor_mul(otile[:, 10 + m, :], ps[:], gtile[:, 2, :])

                # scalar blocks last (leaky relu is not on the critical path)
                for ob in range(4):
                    ps = pspool.tile([128, B], f32)
                    for k in range(4):
                        nc.tensor.matmul(
                            ps[:],
                            w0t[:, k, ob * 128:(ob + 1) * 128],
                            xtile[:, k, :],
                            start=(k == 0),
                            stop=(k == 3),
                        )
                    nc.scalar.activation(otile[:, ob, :], ps[:], AFT.Lrelu, alpha=0.01)

                # outputs drain via the (otherwise idle) GpSimd queue so they
                # never block input prefetch on the Sync ring; the v/t half is
                # ready well before the leaky-relu half
                nc.gpsimd.dma_start(out[:, it, 4:15, :], otile[:, 4:15, :])
                nc.gpsimd.dma_start(out[:, it, 0:4, :], otile[:, 0:4, :])

    nc.compile()
    _compiled_nc = nc
    return nc


def kernel(x, W0, W1, W2):
    global _LAST_RESULTS
    from concourse.bass_utils import run_bass_kernel_spmd

    iodt = np.float16 if _IO16 else np.float32
    x = np.asarray(x, dtype=np.float32)
    W0 = np.asarray(W0, dtype=np.float32)
    W1 = np.asarray(W1, dtype=np.float32)
    W2 = np.asarray(W2, dtype=np.float32)

    nc = _build()
    perm = _perm()

    # transposed + grouped + padded input: [D, 8*NC_PAD]
    xg = np.zeros((D, N_CORES * NC_PAD), dtype=np.float32)
    xg[:, :N_FULL] = x.T[perm]

    w0s = (W0 * np.float32(1.0 / np.sqrt(512.0))).astype(iodt)
    w1s = (W1 * np.float32(1.0 / np.sqrt(256.0))).astype(iodt)
    w2s = (W2 * np.float32(1.0 / np.sqrt(128.0))).astype(iodt)

    in_maps = []
    for c in range(N_CORES):
        xc = xg[:, c * NC_PAD:(c + 1) * NC_PAD]
        # pack to [p, it, chunk, j]: xc[(chunk*128+p), (it*B+j)]
        xp = np.ascontiguousarray(
            xc.reshape(NCHUNK, 128, NT, B).transpose(1, 2, 0, 3).astype(iodt)
        )
        in_maps.append({"xt": xp, "w0": w0s, "w1": w1s, "w2": w2s})

    kwargs = {}
    if _TRACE:
        kwargs["trace"] = True
    res = run_bass_kernel_spmd(nc, in_maps, list(range(N_CORES)), **kwargs)
    _LAST_RESULTS = res

    outg = np.empty((D, N_FULL), dtype=np.float32)
    for c in range(N_CORES):
        oc = res.results[c]["out"]  # [128, NT, NCHUNK, B]
        lo = c * NC_PAD
        hi = min((c + 1) * NC_PAD, N_FULL)
        if hi <= lo:
            continue
        full = oc.transpose(2, 0, 1, 3).reshape(D, NC_PAD).astype(np.float32)
        outg[:, lo:hi] = full[:, :hi - lo]
    out = np.empty((N_FULL, D), dtype=np.float32)
    out[:, perm] = outg.T
    return out


# revision 15
# speedup vs baseline: 1.0146x; 1.0146x over previous
"""Trainium2 Bass kernel for the EquivariantMLPBlock problem.

Math (per row n of x [N, 1920]):
  s = x[:, :512]; v = x[:, 512:1280] as [256, 3]; t = x[:, 1280:] as [128, 5]
  s_out = s @ W0 / sqrt(512)                     -> [896]
  v_out[o, m] = sum_i v[i, m] W1[i, o] / sqrt(256)
  t_out[o, m] = sum_i t[i, m] W2[i, o] / sqrt(128)
  out = [leaky_relu(s_out[:512]),
         (v_out * sigmoid(s_out[512:768])[:, None]).flat,
         (t_out * sigmoid(s_out[768:])[:, None]).flat]

Strategy: data-parallel over rows (8 cores). On the host, the feature
axis is permuted to a "grouped" layout (each m-component of v/t made
contiguous) and x is transposed so features sit on SBUF partitions,
making every matmul a plain weight-stationary PE matmul with rows
streaming on the free axis. The DRAM image is packed per SBUF partition
([p, tile, chunk, col]) so each DMA moves one long contiguous run per
partition (13KB packets instead of 0.9KB rows). Device I/O and matmul
operands are fp16 (halves the DMA bytes; PE runs fp16 at full rate) with
fp32 PSUM accumulation; gating/sigmoid/leaky-relu all run in fp32 on
ACT/DVE. Measured end-to-end error vs the fp32 reference is ~5e-4 of
the output scale (rms ~3.7e-4). Set _IO16=False for fp32r matmuls
(~1.5e-4, ~294us) or _IO16=False,_F32R=False for full fp32 (~1e-6,
~499us). Gate blocks are computed first (their sigmoid feeds every
gating mul), leaky-relu blocks last; outputs drain via the idle GpSimd
DMA queue so stores never block input prefetch on the Sync ring.
Output comes back transposed+grouped and is un-permuted on the host.
"""
import sys
sys.path.insert(0, '/opt/trn_rl_repo')

import numpy as np
from contextlib import ExitStack

D = 1920                 # feature dim
NCHUNK = D // 128        # 15 partition chunks
N_FULL = 50000
N_CORES = 8
NC_PAD = 6272            # rows per core after padding: 8*6272 = 50176
# variable column tiles: a small first tile so compute starts early, then
# 512-row tiles (one PSUM bank each, 15.4KB DMA runs): 128 + 12*512 = 6272
TILE_SIZES = [128] + [512] * 12

_TRACE = False           # set by test harness to capture an NTFF profile
_LAST_RESULTS = None     # stashed BassKernelResults for the harness
_F32R = True             # fp32r matmuls (TF32-like, ~1.5e-4 rel err, 4x PE speed)
_IO16 = True             # fp16 device I/O + fp16 matmul operands (halves DMA bytes)


def _perm():
    # grouped feature order: [s(512) | v m=0 (256) | v m=1 | v m=2 | t m=0 (128) ... t m=4]
    p = list(range(512))
    for m in range(3):
        p += [512 + i * 3 + m for i in range(256)]
    for m in range(5):
        p += [1280 + i * 5 + m for i in range(128)]
    return np.asarray(p, dtype=np.int64)


_compiled_nc = None


def _build():
    global _compiled_nc
    if _compiled_nc is not None:
        return _compiled_nc
    import concourse.tile as tile
    from concourse import bacc, mybir
    from concourse.alu_op_type import AluOpType

    f32 = mybir.dt.float32
    f16 = mybir.dt.float16
    fio = f16 if _IO16 else f32
    fmm = f16 if _IO16 else (mybir.dt.float32r if _F32R else f32)
    AFT = mybir.ActivationFunctionType

    nc = bacc.Bacc("TRN2", target_bir_lowering=False, debug=False)
    # packed flat layout per partition: for each tile (rows r0..r0+bs) the
    # run [r0*NCHUNK : (r0+bs)*NCHUNK] holds [chunk, j] row-major
    TOT = NC_PAD * NCHUNK
    xt = nc.dram_tensor("xt", [128, TOT], fio, kind="ExternalInput").ap()
    w0 = nc.dram_tensor("w0", [512, 896], fio, kind="ExternalInput").ap()
    w1 = nc.dram_tensor("w1", [256, 256], fio, kind="ExternalInput").ap()
    w2 = nc.dram_tensor("w2", [128, 128], fio, kind="ExternalInput").ap()
    out = nc.dram_tensor("out", [128, TOT], fio, kind="ExternalOutput").ap()

    with tile.TileContext(nc) as tc:
        with ExitStack() as ctx:
            wpool = ctx.enter_context(tc.tile_pool(name="w", bufs=1))
            xpool = ctx.enter_context(tc.tile_pool(name="x", bufs=5))
            gpool = ctx.enter_context(tc.tile_pool(name="g", bufs=3))
            opool = ctx.enter_context(tc.tile_pool(name="o", bufs=5))
            pspool = ctx.enter_context(tc.tile_pool(name="ps", bufs=8, space="PSUM"))

            w0t = wpool.tile([128, 4, 896], fmm)
            for k in range(4):
                nc.sync.dma_start(w0t[:, k, :], w0[k * 128:(k + 1) * 128, :].bitcast(fmm))
            w1t = wpool.tile([128, 2, 256], fmm)
            for k in range(2):
                nc.sync.dma_start(w1t[:, k, :], w1[k * 128:(k + 1) * 128, :].bitcast(fmm))
            w2t = wpool.tile([128, 128], fmm)
            nc.sync.dma_start(w2t[:], w2[:, :].bitcast(fmm))

            off = 0
            for bsz in TILE_SIZES:
                flat = slice(off * NCHUNK, (off + bsz) * NCHUNK)
                xtile = xpool.tile([128, NCHUNK, bsz], fmm, tag="xtile")
                nc.sync.dma_start(xtile[:, :, :], xt[:, flat].bitcast(fmm))
                otile = opool.tile([128, NCHUNK, bsz], fio, tag="otile")
                gtile = gpool.tile([128, 3, bsz], f32, tag="gtile")

                # gate blocks first: their sigmoid output feeds every v/t
                # gating mul, so they head the per-tile critical path
                for ob in range(4, 7):
                    ps = pspool.tile([128, bsz], f32, tag="ps")
                    for k in range(4):
                        nc.tensor.matmul(
                            ps[:],
                            w0t[:, k, ob * 128:(ob + 1) * 128],
                            xtile[:, k, :],
                            start=(k == 0),
                            stop=(k == 3),
                        )
                    nc.scalar.activation(gtile[:, ob - 4, :], ps[:], AFT.Sigmoid)

                # 1o block: 3 m-components, each [256 -> 256]
                for m in range(3):
                    for ob in range(2):
                        ps = pspool.tile([128, bsz], f32, tag="ps")
                        for k in range(2):
                            nc.tensor.matmul(
                                ps[:],
                                w1t[:, k, ob * 128:(ob + 1) * 128],
                                xtile[:, 4 + 2 * m + k, :],
                                start=(k == 0),
                                stop=(k == 1),
                            )
                        nc.vector.tensor_mul(otile[:, 4 + 2 * m + ob, :], ps[:], gtile[:, ob, :])

                # 2e block: 5 m-components, each [128 -> 128]
                for m in range(5):
                    ps = pspool.tile([128, bsz], f32, tag="ps")
                    nc.tensor.matmul(ps[:], w2t[:], xtile[:, 10 + m, :], start=True, stop=True)
                    nc.vector.tensor_mul(otile[:, 10 + m, :], ps[:], gtile[:, 2, :])

                # scalar blocks last (leaky relu is not on the critical path)
                for ob in range(4):
                    ps = pspool.tile([128, bsz], f32, tag="ps")
                    for k in range(4):
                        nc.tensor.matmul(
                            ps[:],
                            w0t[:, k, ob * 128:(ob + 1) * 128],
                            xtile[:, k, :],
                            start=(k == 0),
                            stop=(k == 3),
                        )
                    nc.scalar.activation(otile[:, ob, :], ps[:], AFT.Lrelu, alpha=0.01)

                # outputs drain via the (otherwise idle) GpSimd queue so they
                # never block input prefetch on the Sync ring; the v/t half is
                # ready well before the leaky-relu half
                base = off * NCHUNK
                nc.gpsimd.dma_start(
                    out[:, base + 4 * bsz:base + NCHUNK * bsz], otile[:, 4:15, :]
                )
                nc.gpsimd.dma_start(
                    out[:, base:base + 4 * bsz], otile[:, 0:4, :]
                )
                off += bsz

    nc.compile()
    _compiled_nc = nc
    return nc


def kernel(x, W0, W1, W2):
    global _LAST_RESULTS
    from concourse.bass_utils import run_bass_kernel_spmd

    iodt = np.float16 if _IO16 else np.float32
    x = np.asarray(x, dtype=np.float32)
    W0 = np.asarray(W0, dtype=np.float32)
    W1 = np.asarray(W1, dtype=np.float32)
    W2 = np.asarray(W2, dtype=np.float32)

    nc = _build()
    perm = _perm()

    # transposed + grouped + padded input: [D, 8*NC_PAD]
    xg = np.zeros((D, N_CORES * NC_PAD), dtype=np.float32)
    xg[:, :N_FULL] = x.T[perm]

    w0s = (W0 * np.float32(1.0 / np.sqrt(512.0))).astype(iodt)
    w1s = (W1 * np.float32(1.0 / np.sqrt(256.0))).astype(iodt)
    w2s = (W2 * np.float32(1.0 / np.sqrt(128.0))).astype(iodt)

    in_maps = []
    for c in range(N_CORES):
        xc = xg[:, c * NC_PAD:(c + 1) * NC_PAD]
        pieces = []
        off = 0
        for bs in TILE_SIZES:
            pieces.append(
                xc[:, off:off + bs].reshape(NCHUNK, 128, bs)
                .transpose(1, 0, 2).reshape(128, NCHUNK * bs)
            )
            off += bs
        xp = np.ascontiguousarray(np.concatenate(pieces, axis=1).astype(iodt))
        in_maps.append({"xt": xp, "w0": w0s, "w1": w1s, "w2": w2s})

    kwargs = {}
    if _TRACE:
        kwargs["trace"] = True
    res = run_bass_kernel_spmd(nc, in_maps, list(range(N_CORES)), **kwargs)
    _LAST_RESULTS = res

    outg = np.empty((D, N_FULL), dtype=np.float32)
    for c in range(N_CORES):
        oc = res.results[c]["out"]  # [128, NC_PAD*NCHUNK] flat
        lo = c * NC_PAD
        hi = min((c + 1) * NC_PAD, N_FULL)
        if hi <= lo:
            continue
        full = np.empty((D, NC_PAD), dtype=np.float32)
        off = 0
        for bs in TILE_SIZES:
            piece = oc[:, off * NCHUNK:(off + bs) * NCHUNK]
            full[:, off:off + bs] = (
                piece.reshape(128, NCHUNK, bs).transpose(1, 0, 2).reshape(D, bs)
            )
            off += bs
        outg[:, lo:hi] = full[:, :hi - lo]
    out = np.empty((N_FULL, D), dtype=np.float32)
    out[:, perm] = outg.T
    return out
